# revision 21
# baseline (speedup 1.0000x reference)
"""Trainium2 Bass kernel for a Neural ODE (tanh-MLP vector field, Heun/RK2).

Reference computation (per batch row y of width D=512):
    f(y) = tanh(y @ W1 + b1) @ W2 + b2          (H = 2048)
    10 Heun steps, dt = 0.1:
        k1 = f(y); k2 = f(y + dt*k1); y <- y + dt/2*(k1 + k2)

Sharding: data-parallel over the batch axis across 8 NeuronCores
(y0 [8192,512] -> 8 x [1024,512]); weights replicated.

Per-core layout: the state lives TRANSPOSED (y.T, [D, B_local] with D on
partitions) so both matmuls of the MLP chain need no on-chip transposes:
    h.T = W1.T @ y.T   (lhsT = W1 [K=D, M=H],  rhs = y.T  [K=D, N=B])
    z.T = W2.T @ ht.T  (lhsT = W2 [K=H, M=D],  rhs = ht.T [K=H, N=B])
The batch-major <-> feature-major layout conversion is done host-side in
numpy (a few ms on 8+8 MB), so the device runs a pure matmul pipeline.
Matmul operands are stored as float32r (FP22), which streams at 1
cycle/row (full bf16 rate) with fp32 PSUM accumulation.

The batch (N) axis is processed as two 512-wide chunks whose matmuls are
emitted as back-to-back pairs sharing the same stationary weights, and
walrus is run with --enable-ldw-opt=true so the duplicate LDWEIGHTS of
each pair is elided, hiding the weight-load time entirely.
"""

import numpy as np

import concourse.bacc as bacc
import concourse.bass_utils as _bass_utils
import concourse.mybir as mybir
import concourse.tile as tile
from concourse.bass_utils import run_bass_kernel_spmd

# Elide back-to-back LDWEIGHTS of identical weights (our matmul pairs).
if not getattr(_bass_utils, "_ldw_opt_patched", False):
    _orig_run_command = _bass_utils.run_command

    def _run_command_ldw_opt(argv, **kwargs):
        argv = ["--enable-ldw-opt=true" if a == "--enable-ldw-opt=false" else a
                for a in argv]
        return _orig_run_command(argv, **kwargs)

    _bass_utils.run_command = _run_command_ldw_opt
    _bass_utils._ldw_opt_patched = True

N_CORES = 8
BATCH, D, H = 8192, 512, 2048
B = BATCH // N_CORES          # local batch per core: 1024
DT = 0.1
N_STEPS = 10
P = 128
F32 = mybir.dt.float32
F32R = mybir.dt.float32r

D_T = D // P                  # 4  k-tiles / d-tiles
H_T = H // P                  # 16 h-tiles
NCHUNK = 2                    # batch chunks per core (N=512 per matmul)
NW = B // NCHUNK              # 512

_NC_CACHE = None


def _build():
    nc = bacc.Bacc("TRN2", target_bir_lowering=False, debug=False)
    # y0t / outt are the batch shard pre-transposed to [D, B] on the host.
    y0t = nc.dram_tensor("y0t", [D, B], F32, kind="ExternalInput").ap()
    W1 = nc.dram_tensor("W1", [D, H], F32, kind="ExternalInput").ap()
    b1 = nc.dram_tensor("b1", [H], F32, kind="ExternalInput").ap()
    W2 = nc.dram_tensor("W2", [H, D], F32, kind="ExternalInput").ap()
    b2 = nc.dram_tensor("b2", [D], F32, kind="ExternalInput").ap()
    outt = nc.dram_tensor("outt", [D, B], F32, kind="ExternalOutput").ap()

    TANH = mybir.ActivationFunctionType.Tanh
    MULT = mybir.AluOpType.mult
    ADD = mybir.AluOpType.add
    HALF_DT = DT / 2.0

    with tile.TileContext(nc) as tc:
        with (
            tc.tile_pool(name="persist", bufs=1) as persist,
            tc.tile_pool(name="ps_h", bufs=4, space="PSUM") as ps_h_pool,
            tc.tile_pool(name="ps_z", bufs=4, space="PSUM") as ps_z_pool,
        ):
            # Persistent SBUF residents (per-partition bytes in parens).
            w1_k = [persist.tile([P, H], F32R, tag=f"w1k{kt}", name=f"w1k{kt}")
                    for kt in range(D_T)]                        # 32K
            w2_k = [persist.tile([P, D], F32R, tag=f"w2k{kt}", name=f"w2k{kt}")
                    for kt in range(H_T)]                        # 32K
            b1_sb = persist.tile([P, H_T], F32, tag="b1")
            b2_sb = persist.tile([P, D_T], F32, tag="b2")
            y_sb = persist.tile([P, D_T * B], F32R, tag="y")     # 16K
            y_acc = persist.tile([P, D_T * B], F32, tag="yacc")  # 16K
            y_mid = persist.tile([P, D_T * B], F32R, tag="ymid")  # 16K
            ht_sb = persist.tile([P, H_T * B], F32R, tag="ht")   # 64K

            # --- input DMAs, in consumption order ---
            for kt in range(D_T):
                nc.sync.dma_start(y_sb[:, kt * B:(kt + 1) * B],
                                  y0t[kt * P:(kt + 1) * P, :].bitcast(F32R))
            for kt in range(D_T):
                nc.sync.dma_start(w1_k[kt][:],
                                  W1[kt * P:(kt + 1) * P, :].bitcast(F32R))
            nc.sync.dma_start(b1_sb[:], b1.rearrange("(m p) -> p m", p=P))
            nc.sync.dma_start(b2_sb[:], b2.rearrange("(m p) -> p m", p=P))
            for kt in range(H_T):
                nc.sync.dma_start(w2_k[kt][:],
                                  W2[kt * P:(kt + 1) * P, :].bitcast(F32R))

            def feval(X, consume):
                """One vector-field evaluation: z.T = W2.T@tanh(W1.T@X + b1).

                X: SBUF state tile [P, D_T*B] holding X.T; consume(dm, n0, pz)
                receives each z.T output PSUM tile [P, NW] (pre-b2).
                Both batch chunks advance together as weight-sharing matmul
                pairs.
                """
                for m in range(H_T):
                    ph = [ps_h_pool.tile([P, NW], F32, tag="ps_h", name="ph")
                          for _ in range(NCHUNK)]
                    for kt in range(D_T):
                        w_ap = w1_k[kt][:, m * P:(m + 1) * P]
                        for c in range(NCHUNK):
                            nc.tensor.matmul(
                                ph[c][:], w_ap,
                                X[:, kt * B + c * NW: kt * B + c * NW + NW],
                                start=(kt == 0), stop=(kt == D_T - 1))
                    for c in range(NCHUNK):
                        nc.scalar.activation(
                            ht_sb[:, m * B + c * NW: m * B + (c + 1) * NW],
                            ph[c][:], TANH, bias=b1_sb[:, m:m + 1])
                for dm in range(D_T):
                    pz = [ps_z_pool.tile([P, NW], F32, tag="ps_z", name="pz")
                          for _ in range(NCHUNK)]
                    for kt in range(H_T):
                        w_ap = w2_k[kt][:, dm * P:(dm + 1) * P]
                        for c in range(NCHUNK):
                            nc.tensor.matmul(
                                pz[c][:], w_ap,
                                ht_sb[:, kt * B + c * NW: kt * B + c * NW + NW],
                                start=(kt == 0), stop=(kt == H_T - 1))
                    for c in range(NCHUNK):
                        consume(dm, c * NW, pz[c])

            def consume_k1(dm, n0, pz):
                off = dm * B + n0
                # z -> k1 = z + b2 ; y_mid = y + dt*k1 ; y_acc = y + dt/2*k1
                nc.vector.tensor_scalar_add(pz[:], pz[:], b2_sb[:, dm:dm + 1])
                nc.vector.scalar_tensor_tensor(
                    y_mid[:, off:off + NW], pz[:], DT, y_sb[:, off:off + NW],
                    op0=MULT, op1=ADD)
                nc.vector.scalar_tensor_tensor(
                    y_acc[:, off:off + NW], pz[:], HALF_DT, y_sb[:, off:off + NW],
                    op0=MULT, op1=ADD)

            def consume_k2(dm, n0, pz):
                off = dm * B + n0
                # y <- y_acc + dt/2*(z + b2)
                nc.vector.tensor_scalar_add(pz[:], pz[:], b2_sb[:, dm:dm + 1])
                nc.vector.scalar_tensor_tensor(
                    y_sb[:, off:off + NW], pz[:], HALF_DT, y_acc[:, off:off + NW],
                    op0=MULT, op1=ADD)

            for _step in range(N_STEPS):
                feval(y_sb, consume_k1)
                feval(y_mid, consume_k2)

            # --- final store: y.T tiles straight out; host re-transposes ---
            for kt in range(D_T):
                nc.sync.dma_start(outt[kt * P:(kt + 1) * P, :],
                                  y_sb[:, kt * B:(kt + 1) * B].bitcast(F32))

    nc.compile()
    return nc


def get_nc():
    global _NC_CACHE
    if _NC_CACHE is None:
        _NC_CACHE = _build()
    return _NC_CACHE


def run(inputs, trace=False, **kwargs):
    nc = get_nc()
    y0 = np.asarray(inputs["y0"], dtype=np.float32)
    W1 = np.ascontiguousarray(np.asarray(inputs["W1"], dtype=np.float32))
    b1 = np.ascontiguousarray(np.asarray(inputs["b1"], dtype=np.float32))
    W2 = np.ascontiguousarray(np.asarray(inputs["W2"], dtype=np.float32))
    b2 = np.ascontiguousarray(np.asarray(inputs["b2"], dtype=np.float32))
    # shard over batch, pre-transpose each shard to [D, B] feature-major
    shards_t = np.ascontiguousarray(
        y0.reshape(N_CORES, B, D).transpose(0, 2, 1))
    in_maps = [{"y0t": shards_t[i], "W1": W1, "b1": b1, "W2": W2, "b2": b2}
               for i in range(N_CORES)]
    res = run_bass_kernel_spmd(nc, in_maps, core_ids=list(range(N_CORES)),
                               trace=trace, **kwargs)
    out_t = np.stack([r["outt"] for r in res.results])      # [8, D, B]
    full = np.ascontiguousarray(
        out_t.transpose(0, 2, 1).reshape(BATCH, D))
    return full, res


def kernel(**inputs) -> np.ndarray:
    full, _ = run(inputs, trace=False)
    return full


# revision 23
# speedup vs baseline: 1.0028x; 1.0028x over previous
"""Trainium2 Bass kernel for a Neural ODE (tanh-MLP vector field, Heun/RK2).

Reference computation (per batch row y of width D=512):
    f(y) = tanh(y @ W1 + b1) @ W2 + b2          (H = 2048)
    10 Heun steps, dt = 0.1:
        k1 = f(y); k2 = f(y + dt*k1); y <- y + dt/2*(k1 + k2)

Sharding: data-parallel over the batch axis across 8 NeuronCores
(y0 [8192,512] -> 8 x [1024,512]); weights replicated.

Per-core layout: the state lives TRANSPOSED (y.T, [D, B_local] with D on
partitions) so both matmuls of the MLP chain need no on-chip transposes:
    h.T = W1.T @ y.T   (lhsT = W1 [K=D, M=H],  rhs = y.T  [K=D, N=B])
    z.T = W2.T @ ht.T  (lhsT = W2 [K=H, M=D],  rhs = ht.T [K=H, N=B])
The batch-major <-> feature-major layout conversion is done host-side in
numpy (a few ms on 8+8 MB), so the device runs a pure matmul pipeline.
Matmul operands are stored as float32r (FP22), which streams at 1
cycle/row (full bf16 rate) with fp32 PSUM accumulation.

The batch (N) axis is processed as two 512-wide chunks whose matmuls are
emitted as back-to-back pairs sharing the same stationary weights, and
walrus is run with --enable-ldw-opt=true so the duplicate LDWEIGHTS of
each pair is elided, hiding the weight-load time entirely.
"""

import numpy as np

import concourse.bacc as bacc
import concourse.bass_utils as _bass_utils
import concourse.mybir as mybir
import concourse.tile as tile
from concourse.bass_utils import run_bass_kernel_spmd

# Elide back-to-back LDWEIGHTS of identical weights (our matmul pairs).
if not getattr(_bass_utils, "_ldw_opt_patched", False):
    _orig_run_command = _bass_utils.run_command

    def _run_command_ldw_opt(argv, **kwargs):
        argv = ["--enable-ldw-opt=true" if a == "--enable-ldw-opt=false" else a
                for a in argv]
        return _orig_run_command(argv, **kwargs)

    _bass_utils.run_command = _run_command_ldw_opt
    _bass_utils._ldw_opt_patched = True

N_CORES = 8
BATCH, D, H = 8192, 512, 2048
B = BATCH // N_CORES          # local batch per core: 1024
DT = 0.1
N_STEPS = 10
P = 128
F32 = mybir.dt.float32
F32R = mybir.dt.float32r

D_T = D // P                  # 4  k-tiles / d-tiles
H_T = H // P                  # 16 h-tiles
NCHUNK = 2                    # batch chunks per core (N=512 per matmul)
NW = B // NCHUNK              # 512

_NC_CACHE = None


def _build():
    nc = bacc.Bacc("TRN2", target_bir_lowering=False, debug=False)
    # y0t / outt are the batch shard pre-transposed to [D, B] on the host.
    y0t = nc.dram_tensor("y0t", [D, B], F32, kind="ExternalInput").ap()
    W1 = nc.dram_tensor("W1", [D, H], F32, kind="ExternalInput").ap()
    b1 = nc.dram_tensor("b1", [H], F32, kind="ExternalInput").ap()
    W2 = nc.dram_tensor("W2", [H, D], F32, kind="ExternalInput").ap()
    b2 = nc.dram_tensor("b2", [D], F32, kind="ExternalInput").ap()
    outt = nc.dram_tensor("outt", [D, B], F32, kind="ExternalOutput").ap()

    TANH = mybir.ActivationFunctionType.Tanh
    MULT = mybir.AluOpType.mult
    ADD = mybir.AluOpType.add
    HALF_DT = DT / 2.0

    with tile.TileContext(nc) as tc:
        with (
            tc.tile_pool(name="persist", bufs=1) as persist,
            tc.tile_pool(name="ps_h", bufs=4, space="PSUM") as ps_h_pool,
            tc.tile_pool(name="ps_z", bufs=4, space="PSUM") as ps_z_pool,
        ):
            # Persistent SBUF residents (per-partition bytes in parens).
            w1_k = [persist.tile([P, H], F32R, tag=f"w1k{kt}", name=f"w1k{kt}")
                    for kt in range(D_T)]                        # 32K
            w2_k = [persist.tile([P, D], F32R, tag=f"w2k{kt}", name=f"w2k{kt}")
                    for kt in range(H_T)]                        # 32K
            b1_sb = persist.tile([P, H_T], F32, tag="b1")
            b2_sb = persist.tile([P, D_T], F32, tag="b2")
            y_sb = persist.tile([P, D_T * B], F32R, tag="y")     # 16K
            y_acc = persist.tile([P, D_T * B], F32, tag="yacc")  # 16K
            y_mid = persist.tile([P, D_T * B], F32R, tag="ymid")  # 16K
            ht_sb = persist.tile([P, H_T * B], F32R, tag="ht")   # 64K

            # --- input DMAs, in consumption order ---
            for kt in range(D_T):
                nc.sync.dma_start(y_sb[:, kt * B:(kt + 1) * B],
                                  y0t[kt * P:(kt + 1) * P, :].bitcast(F32R))
            for kt in range(D_T):
                nc.sync.dma_start(w1_k[kt][:],
                                  W1[kt * P:(kt + 1) * P, :].bitcast(F32R))
            nc.sync.dma_start(b1_sb[:], b1.rearrange("(m p) -> p m", p=P))
            nc.sync.dma_start(b2_sb[:], b2.rearrange("(m p) -> p m", p=P))
            for kt in range(H_T):
                nc.sync.dma_start(w2_k[kt][:],
                                  W2[kt * P:(kt + 1) * P, :].bitcast(F32R))

            # PE warm-up with NO DMA dependency: dummy matmuls on an on-chip
            # memset scratch keep the HAM activity window filled from the end
            # of the prologue until the input DMAs land, so the real stream
            # starts at 2.4 GHz. Sized to end just before the critical
            # y+w1 set lands (~19us); worst-case delay is one dummy (~0.4us).
            scratch = persist.tile([P, NW], F32R, tag="scratch")
            nc.vector.memset(y_acc[:, 0:NW], 0.0)
            nc.vector.tensor_copy(scratch[:], y_acc[:, 0:NW])
            warm_ps = ps_h_pool.tile([P, NW], F32, tag="ps_h", name="warm_ps")
            for _ in range(40):
                nc.tensor.matmul(warm_ps[:], scratch[:, 0:P], scratch[:],
                                 start=True, stop=True)

            def feval(X, consume):
                """One vector-field evaluation: z.T = W2.T@tanh(W1.T@X + b1).

                X: SBUF state tile [P, D_T*B] holding X.T; consume(dm, n0, pz)
                receives each z.T output PSUM tile [P, NW] (pre-b2).
                Both batch chunks advance together as weight-sharing matmul
                pairs.
                """
                for m in range(H_T):
                    ph = [ps_h_pool.tile([P, NW], F32, tag="ps_h", name="ph")
                          for _ in range(NCHUNK)]
                    for kt in range(D_T):
                        w_ap = w1_k[kt][:, m * P:(m + 1) * P]
                        for c in range(NCHUNK):
                            nc.tensor.matmul(
                                ph[c][:], w_ap,
                                X[:, kt * B + c * NW: kt * B + c * NW + NW],
                                start=(kt == 0), stop=(kt == D_T - 1))
                    for c in range(NCHUNK):
                        nc.scalar.activation(
                            ht_sb[:, m * B + c * NW: m * B + (c + 1) * NW],
                            ph[c][:], TANH, bias=b1_sb[:, m:m + 1])
                for dm in range(D_T):
                    pz = [ps_z_pool.tile([P, NW], F32, tag="ps_z", name="pz")
                          for _ in range(NCHUNK)]
                    for kt in range(H_T):
                        w_ap = w2_k[kt][:, dm * P:(dm + 1) * P]
                        for c in range(NCHUNK):
                            nc.tensor.matmul(
                                pz[c][:], w_ap,
                                ht_sb[:, kt * B + c * NW: kt * B + c * NW + NW],
                                start=(kt == 0), stop=(kt == H_T - 1))
                    for c in range(NCHUNK):
                        consume(dm, c * NW, pz[c])

            def consume_k1(dm, n0, pz):
                off = dm * B + n0
                # z -> k1 = z + b2 ; y_mid = y + dt*k1 ; y_acc = y + dt/2*k1
                nc.vector.tensor_scalar_add(pz[:], pz[:], b2_sb[:, dm:dm + 1])
                nc.vector.scalar_tensor_tensor(
                    y_mid[:, off:off + NW], pz[:], DT, y_sb[:, off:off + NW],
                    op0=MULT, op1=ADD)
                nc.vector.scalar_tensor_tensor(
                    y_acc[:, off:off + NW], pz[:], HALF_DT, y_sb[:, off:off + NW],
                    op0=MULT, op1=ADD)

            def consume_k2(dm, n0, pz):
                off = dm * B + n0
                # y <- y_acc + dt/2*(z + b2)
                nc.vector.tensor_scalar_add(pz[:], pz[:], b2_sb[:, dm:dm + 1])
                nc.vector.scalar_tensor_tensor(
                    y_sb[:, off:off + NW], pz[:], HALF_DT, y_acc[:, off:off + NW],
                    op0=MULT, op1=ADD)

            for _step in range(N_STEPS):
                feval(y_sb, consume_k1)
                feval(y_mid, consume_k2)

            # --- final store: y.T tiles straight out; host re-transposes ---
            for kt in range(D_T):
                nc.sync.dma_start(outt[kt * P:(kt + 1) * P, :],
                                  y_sb[:, kt * B:(kt + 1) * B].bitcast(F32))

    nc.compile()
    return nc


def get_nc():
    global _NC_CACHE
    if _NC_CACHE is None:
        _NC_CACHE = _build()
    return _NC_CACHE


def run(inputs, trace=False, **kwargs):
    nc = get_nc()
    y0 = np.asarray(inputs["y0"], dtype=np.float32)
    W1 = np.ascontiguousarray(np.asarray(inputs["W1"], dtype=np.float32))
    b1 = np.ascontiguousarray(np.asarray(inputs["b1"], dtype=np.float32))
    W2 = np.ascontiguousarray(np.asarray(inputs["W2"], dtype=np.float32))
    b2 = np.ascontiguousarray(np.asarray(inputs["b2"], dtype=np.float32))
    # shard over batch, pre-transpose each shard to [D, B] feature-major
    shards_t = np.ascontiguousarray(
        y0.reshape(N_CORES, B, D).transpose(0, 2, 1))
    in_maps = [{"y0t": shards_t[i], "W1": W1, "b1": b1, "W2": W2, "b2": b2}
               for i in range(N_CORES)]
    res = run_bass_kernel_spmd(nc, in_maps, core_ids=list(range(N_CORES)),
                               trace=trace, **kwargs)
    out_t = np.stack([r["outt"] for r in res.results])      # [8, D, B]
    full = np.ascontiguousarray(
        out_t.transpose(0, 2, 1).reshape(BATCH, D))
    return full, res


def kernel(**inputs) -> np.ndarray:
    full, _ = run(inputs, trace=False)
    return full


# revision 24
# speedup vs baseline: 1.0079x; 1.0051x over previous
"""Trainium2 Bass kernel for a Neural ODE (tanh-MLP vector field, Heun/RK2).

Reference computation (per batch row y of width D=512):
    f(y) = tanh(y @ W1 + b1) @ W2 + b2          (H = 2048)
    10 Heun steps, dt = 0.1:
        k1 = f(y); k2 = f(y + dt*k1); y <- y + dt/2*(k1 + k2)

Sharding: data-parallel over the batch axis across 8 NeuronCores
(y0 [8192,512] -> 8 x [1024,512]); weights replicated.

Per-core layout: the state lives TRANSPOSED (y.T, [D, B_local] with D on
partitions) so both matmuls of the MLP chain need no on-chip transposes:
    h.T = W1.T @ y.T   (lhsT = W1 [K=D, M=H],  rhs = y.T  [K=D, N=B])
    z.T = W2.T @ ht.T  (lhsT = W2 [K=H, M=D],  rhs = ht.T [K=H, N=B])
The batch-major <-> feature-major layout conversion is done host-side in
numpy (a few ms on 8+8 MB), so the device runs a pure matmul pipeline.
Matmul operands are stored as float32r (FP22), which streams at 1
cycle/row (full bf16 rate) with fp32 PSUM accumulation.

The batch (N) axis is processed as two 512-wide chunks whose matmuls are
emitted as back-to-back pairs sharing the same stationary weights, and
walrus is run with --enable-ldw-opt=true so the duplicate LDWEIGHTS of
each pair is elided, hiding the weight-load time entirely.
"""

import numpy as np

import concourse.bacc as bacc
import concourse.bass_utils as _bass_utils
import concourse.mybir as mybir
import concourse.tile as tile
from concourse.bass_utils import run_bass_kernel_spmd

# Elide back-to-back LDWEIGHTS of identical weights (our matmul pairs).
if not getattr(_bass_utils, "_ldw_opt_patched", False):
    _orig_run_command = _bass_utils.run_command

    def _run_command_ldw_opt(argv, **kwargs):
        argv = ["--enable-ldw-opt=true" if a == "--enable-ldw-opt=false" else a
                for a in argv]
        return _orig_run_command(argv, **kwargs)

    _bass_utils.run_command = _run_command_ldw_opt
    _bass_utils._ldw_opt_patched = True

N_CORES = 8
BATCH, D, H = 8192, 512, 2048
B = BATCH // N_CORES          # local batch per core: 1024
DT = 0.1
N_STEPS = 10
P = 128
F32 = mybir.dt.float32
F32R = mybir.dt.float32r

D_T = D // P                  # 4  k-tiles / d-tiles
H_T = H // P                  # 16 h-tiles
NCHUNK = 2                    # batch chunks per core (N=512 per matmul)
NW = B // NCHUNK              # 512

_NC_CACHE = None


def _build():
    nc = bacc.Bacc("TRN2", target_bir_lowering=False, debug=False)
    # y0t / outt are the batch shard pre-transposed to [D, B] on the host.
    y0t = nc.dram_tensor("y0t", [D, B], F32, kind="ExternalInput").ap()
    W1 = nc.dram_tensor("W1", [D, H], F32, kind="ExternalInput").ap()
    b1 = nc.dram_tensor("b1", [H], F32, kind="ExternalInput").ap()
    W2 = nc.dram_tensor("W2", [H, D], F32, kind="ExternalInput").ap()
    b2 = nc.dram_tensor("b2", [D], F32, kind="ExternalInput").ap()
    outt = nc.dram_tensor("outt", [D, B], F32, kind="ExternalOutput").ap()

    TANH = mybir.ActivationFunctionType.Tanh
    MULT = mybir.AluOpType.mult
    ADD = mybir.AluOpType.add
    HALF_DT = DT / 2.0

    with tile.TileContext(nc) as tc:
        with (
            tc.tile_pool(name="persist", bufs=1) as persist,
            tc.tile_pool(name="ps_h", bufs=4, space="PSUM") as ps_h_pool,
            tc.tile_pool(name="ps_z", bufs=4, space="PSUM") as ps_z_pool,
        ):
            # Persistent SBUF residents (per-partition bytes in parens).
            w1_k = [persist.tile([P, H], F32R, tag=f"w1k{kt}", name=f"w1k{kt}")
                    for kt in range(D_T)]                        # 32K
            w2_k = [persist.tile([P, D], F32R, tag=f"w2k{kt}", name=f"w2k{kt}")
                    for kt in range(H_T)]                        # 32K
            b1_sb = persist.tile([P, H_T], F32, tag="b1")
            b2_sb = persist.tile([P, D_T], F32, tag="b2")
            y_sb = persist.tile([P, D_T * B], F32R, tag="y")     # 16K
            y_acc = persist.tile([P, D_T * B], F32, tag="yacc")  # 16K
            y_mid = persist.tile([P, D_T * B], F32R, tag="ymid")  # 16K
            ht_sb = persist.tile([P, H_T * B], F32R, tag="ht")   # 64K

            # --- input DMAs, in consumption order ---
            for kt in range(D_T):
                nc.sync.dma_start(y_sb[:, kt * B:(kt + 1) * B],
                                  y0t[kt * P:(kt + 1) * P, :].bitcast(F32R))
            for kt in range(D_T):
                nc.sync.dma_start(w1_k[kt][:],
                                  W1[kt * P:(kt + 1) * P, :].bitcast(F32R))
            nc.sync.dma_start(b1_sb[:], b1.rearrange("(m p) -> p m", p=P))
            nc.sync.dma_start(b2_sb[:], b2.rearrange("(m p) -> p m", p=P))
            for kt in range(H_T):
                nc.sync.dma_start(w2_k[kt][:],
                                  W2[kt * P:(kt + 1) * P, :].bitcast(F32R))

            def feval(X, consume):
                """One vector-field evaluation: z.T = W2.T@tanh(W1.T@X + b1).

                X: SBUF state tile [P, D_T*B] holding X.T; consume(dm, n0, pz)
                receives each z.T output PSUM tile [P, NW] (pre-b2).
                Both batch chunks advance together as weight-sharing matmul
                pairs.
                """
                for m in range(H_T):
                    ph = [ps_h_pool.tile([P, NW], F32, tag="ps_h", name="ph")
                          for _ in range(NCHUNK)]
                    for kt in range(D_T):
                        w_ap = w1_k[kt][:, m * P:(m + 1) * P]
                        for c in range(NCHUNK):
                            nc.tensor.matmul(
                                ph[c][:], w_ap,
                                X[:, kt * B + c * NW: kt * B + c * NW + NW],
                                start=(kt == 0), stop=(kt == D_T - 1))
                    for c in range(NCHUNK):
                        nc.scalar.activation(
                            ht_sb[:, m * B + c * NW: m * B + (c + 1) * NW],
                            ph[c][:], TANH, bias=b1_sb[:, m:m + 1])
                for dm in range(D_T):
                    pz = [ps_z_pool.tile([P, NW], F32, tag="ps_z", name="pz")
                          for _ in range(NCHUNK)]
                    for kt in range(H_T):
                        w_ap = w2_k[kt][:, dm * P:(dm + 1) * P]
                        for c in range(NCHUNK):
                            nc.tensor.matmul(
                                pz[c][:], w_ap,
                                ht_sb[:, kt * B + c * NW: kt * B + c * NW + NW],
                                start=(kt == 0), stop=(kt == H_T - 1))
                    for c in range(NCHUNK):
                        consume(dm, c * NW, pz[c])

            def consume_k1(dm, n0, pz):
                off = dm * B + n0
                # z -> k1 = z + b2 ; y_mid = y + dt*k1 ; y_acc = y + dt/2*k1
                nc.vector.tensor_scalar_add(pz[:], pz[:], b2_sb[:, dm:dm + 1])
                nc.vector.scalar_tensor_tensor(
                    y_mid[:, off:off + NW], pz[:], DT, y_sb[:, off:off + NW],
                    op0=MULT, op1=ADD)
                nc.vector.scalar_tensor_tensor(
                    y_acc[:, off:off + NW], pz[:], HALF_DT, y_sb[:, off:off + NW],
                    op0=MULT, op1=ADD)

            def consume_k2(dm, n0, pz):
                off = dm * B + n0
                # y <- y_acc + dt/2*(z + b2)
                nc.vector.tensor_scalar_add(pz[:], pz[:], b2_sb[:, dm:dm + 1])
                nc.vector.scalar_tensor_tensor(
                    y_sb[:, off:off + NW], pz[:], HALF_DT, y_acc[:, off:off + NW],
                    op0=MULT, op1=ADD)

            for _step in range(N_STEPS):
                feval(y_sb, consume_k1)
                feval(y_mid, consume_k2)

            # --- final store: y.T tiles straight out; host re-transposes ---
            for kt in range(D_T):
                nc.sync.dma_start(outt[kt * P:(kt + 1) * P, :],
                                  y_sb[:, kt * B:(kt + 1) * B].bitcast(F32))

    nc.compile()
    return nc


def get_nc():
    global _NC_CACHE
    if _NC_CACHE is None:
        _NC_CACHE = _build()
    return _NC_CACHE


def run(inputs, trace=False, **kwargs):
    nc = get_nc()
    y0 = np.asarray(inputs["y0"], dtype=np.float32)
    W1 = np.ascontiguousarray(np.asarray(inputs["W1"], dtype=np.float32))
    b1 = np.ascontiguousarray(np.asarray(inputs["b1"], dtype=np.float32))
    W2 = np.ascontiguousarray(np.asarray(inputs["W2"], dtype=np.float32))
    b2 = np.ascontiguousarray(np.asarray(inputs["b2"], dtype=np.float32))
    # shard over batch, pre-transpose each shard to [D, B] feature-major
    shards_t = np.ascontiguousarray(
        y0.reshape(N_CORES, B, D).transpose(0, 2, 1))
    in_maps = [{"y0t": shards_t[i], "W1": W1, "b1": b1, "W2": W2, "b2": b2}
               for i in range(N_CORES)]
    res = run_bass_kernel_spmd(nc, in_maps, core_ids=list(range(N_CORES)),
                               trace=trace, **kwargs)
    out_t = np.stack([r["outt"] for r in res.results])      # [8, D, B]
    full = np.ascontiguousarray(
        out_t.transpose(0, 2, 1).reshape(BATCH, D))
    return full, res


def kernel(**inputs) -> np.ndarray:
    full, _ = run(inputs, trace=False)
    return full
